# revision 12
# baseline (speedup 1.0000x reference)
"""Trainium2 Bass kernel for nn_CNILUT: per-pixel MLP (3->256->256->256->256->3)
with relu/tanh activations and residual clamp, data-parallel over 8 NeuronCores.

Strategy:
- Shard the flattened pixel axis (n*h*w = 1,048,576 px) across 8 cores
  (131,072 px each); replicate the tiny MLP weights.
- Feature-major dataflow: activations live as [features(partitions), pixels]
  which is exactly the channel-planar layout of x, so no transposes anywhere.
- style is folded into layer-0's bias on the host (b0_eff = b0 + style@W0[3:6]),
  so layer 0 is a K=3 matmul over the 3 image channels only.
- Matmuls run as float32r (TF32-like, 1 cycle/row vs 4 for fp32; rel err ~2e-4).
- tanh (+bias) on ScalarE directly from PSUM; relu (+bias) and the final
  residual-add + clamp on VectorE.
"""

import os
import sys

for _p in ("/opt/trn_rl_repo", "/root/.axon_site/_ro/trn_rl_repo"):
    if os.path.isdir(_p) and _p not in sys.path:
        sys.path.insert(0, _p)

import numpy as np

import concourse.bass as bass
import concourse.tile as tile
from concourse import mybir
from concourse.bass_utils import run_bass_kernel_spmd

F32 = mybir.dt.float32
F32R = mybir.dt.float32r

N_CORES = 8
N, C, H, W = 4, 3, 512, 512
NF = 256
PXC = (N * H * W) // N_CORES  # pixels per core = 131072
T = 1024                      # pixels per tile
NT = PXC // T                 # 128 tiles per core

# packed weight layout (columns of the [128, WCOLS] f32r "wts" input)
# W{l}k{k} for hidden layers l=1..3 at (l-1)*512 + k*256, each [128, 256]
W4_OFF = 3 * 512              # W4k0 [128,3], W4k1 [128,3]
W0_OFF = W4_OFF + 6           # W0_eff [3, 256] on partitions 0..2
WCOLS = W0_OFF + 256

_CACHE = {}


def _build_module(nt=NT, split_waits=True, detect_races=True, reps=1):
    pxc = nt * T
    nc = bass.Bass(detect_race_conditions=detect_races)
    xg = nc.declare_dram_parameter("xg", [C, pxc], F32R, isOutput=False)
    wts = nc.declare_dram_parameter("wts", [128, WCOLS], F32R, isOutput=False)
    bias = nc.declare_dram_parameter("bias", [128, 9], F32, isOutput=False)
    og = nc.declare_dram_parameter("og", [C, pxc], F32, isOutput=True)

    TANH = mybir.ActivationFunctionType.Tanh
    ADD = mybir.AluOpType.add
    MAX = mybir.AluOpType.max
    MIN = mybir.AluOpType.min

    with tile.TileContext(nc) as tc:
        with tc.tile_pool(name="const", bufs=1) as const, \
             tc.tile_pool(name="io", bufs=3) as io, \
             tc.tile_pool(name="zs", bufs=2) as zs, \
             tc.tile_pool(name="ps", bufs=2, space="PSUM") as ps:
            w_t = const.tile([128, WCOLS], F32R)
            b_t = const.tile([128, 9], F32)
            nc.sync.dma_start(out=w_t[:], in_=wts[:])
            nc.sync.dma_start(out=b_t[:], in_=bias[:])

            def lw(l, k, m):  # lhsT AP for hidden layer l (1..3), k/m chunks
                base = (l - 1) * 512 + k * 256
                return w_t[:, base + 128 * m: base + 128 * (m + 1)]

            for t in [tt for _ in range(reps) for tt in range(nt)]:
                x_t = io.tile([C, T], F32R, tag="x")
                nc.sync.dma_start(out=x_t[:], in_=xg[:, t * T:(t + 1) * T])

                # ---- L0: K=3 matmul, relu(+b0) on VectorE ----
                pA = [ps.tile([128, T], F32, tag="pA", name=f"pA{m}")
                      for m in range(2)]
                for m in range(2):
                    for h in range(2):
                        nc.tensor.matmul(
                            pA[m][:, h * 512:(h + 1) * 512],
                            w_t[0:3, W0_OFF + 128 * m: W0_OFF + 128 * (m + 1)],
                            x_t[:, h * 512:(h + 1) * 512],
                            start=True, stop=True)
                z = [zs.tile([128, T], F32R, tag=f"z{m}", name=f"z{m}")
                     for m in range(2)]
                for m in range(2):
                    nc.vector.tensor_scalar(
                        z[m][:], pA[m][:], b_t[:, m:m + 1], 0.0, ADD, MAX)

                # ---- L1..L3: K=256 f32r matmuls, tanh(+b) on ScalarE ----
                for l in (1, 2, 3):
                    pool_tag = "pB" if l % 2 == 1 else "pA"
                    pN = [ps.tile([128, T], F32, tag=pool_tag, name=f"p{l}{m}")
                          for m in range(2)]
                    for m in range(2):
                        for h in range(2):
                            for k in range(2):
                                nc.tensor.matmul(
                                    pN[m][:, h * 512:(h + 1) * 512],
                                    lw(l, k, m),
                                    z[k][:, h * 512:(h + 1) * 512],
                                    start=(k == 0), stop=(k == 1))
                    zn = [zs.tile([128, T], F32R, tag=f"z{l}{m}", name=f"z{l}{m}")
                          for m in range(2)]
                    for m in range(2):
                        nc.scalar.activation(
                            zn[m][:], pN[m][:], TANH,
                            bias=b_t[:, 2 * l + m:2 * l + m + 1], scale=1.0)
                    z = zn

                # ---- L4: M=3 matmul; residual + b4 + clamp on VectorE ----
                pC = ps.tile([3, T], F32, tag="pA")
                for h in range(2):
                    for k in range(2):
                        nc.tensor.matmul(
                            pC[:, h * 512:(h + 1) * 512],
                            w_t[:, W4_OFF + 3 * k: W4_OFF + 3 * (k + 1)],
                            z[k][:, h * 512:(h + 1) * 512],
                            start=(k == 0), stop=(k == 1))
                o_t = io.tile([C, T], F32, tag="o")
                nc.vector.scalar_tensor_tensor(
                    o_t[:], pC[:], b_t[0:3, 8:9], x_t[:], ADD, ADD)
                nc.vector.tensor_scalar(o_t[:], o_t[:], 0.0, 1.0, MAX, MIN)
                nc.sync.dma_start(out=og[:, t * T:(t + 1) * T], in_=o_t[:])

    if split_waits:
        _split_multi_waits(nc, limit=1)
    return nc


def _split_multi_waits(nc, limit=1):
    """walrus codegen on this toolchain accepts only `limit` sync waits per
    instruction (the fused f32r matmul's LDWEIGHTS slot takes just one).
    Tile attaches N waits freely; split the extras onto single-wait NoOps
    immediately preceding, on the same engine — semantics preserving since
    an engine queue executes in order."""
    n = 0
    for fn in nc.m.functions:
        for bb in fn.blocks:
            insts = bb.instructions
            out = []
            changed = False
            for inst in insts:
                si = inst.sync_info
                if si is not None and si.on_wait and len(si.on_wait) > limit:
                    waits = list(si.on_wait)
                    for j, w in enumerate(waits[:-limit]):
                        nop = mybir.InstNoOp(name=f"{inst.name}-wsplit{j}")
                        nop.engine = inst.engine
                        nop.sync_info = mybir.SyncInfo(on_wait=[w], on_update=[])
                        out.append(nop)
                        n += 1
                    inst.sync_info = mybir.SyncInfo(
                        on_wait=waits[-limit:], on_update=list(si.on_update))
                    changed = True
                out.append(inst)
            if changed:
                insts.clear()
                insts.extend(out)
    return n


def _pack_weights(style, W0, b0, W1, b1, W2, b2, W3, b3, W4, b4):
    w = np.zeros((128, WCOLS), dtype=np.float32)
    for l, Wl in ((1, W1), (2, W2), (3, W3)):
        base = (l - 1) * 512
        w[:, base:base + 256] = Wl[0:128, :]
        w[:, base + 256:base + 512] = Wl[128:256, :]
    w[:, W4_OFF:W4_OFF + 3] = W4[0:128, :]
    w[:, W4_OFF + 3:W4_OFF + 6] = W4[128:256, :]
    w[0:3, W0_OFF:W0_OFF + 256] = W0[0:3, :]

    b0_eff = b0 + style @ W0[3:6, :]
    b = np.zeros((128, 9), dtype=np.float32)
    for i, bl in enumerate((b0_eff, b1, b2, b3)):
        b[:, 2 * i] = bl[0:128]
        b[:, 2 * i + 1] = bl[128:256]
    b[0:3, 8] = b4
    return w, b


def _build_io_baseline():
    """Same external IO as the real kernel, but pure DMA passthrough —
    used by test.py to subtract host<->device transfer overhead from
    wall-clock timings."""
    nc = bass.Bass()
    xg = nc.declare_dram_parameter("xg", [C, PXC], F32R, isOutput=False)
    wts = nc.declare_dram_parameter("wts", [128, WCOLS], F32R, isOutput=False)
    bias = nc.declare_dram_parameter("bias", [128, 9], F32, isOutput=False)
    og = nc.declare_dram_parameter("og", [C, PXC], F32, isOutput=True)
    with tile.TileContext(nc) as tc:
        with tc.tile_pool(name="sb", bufs=2) as sb:
            w_t = sb.tile([128, WCOLS], F32R, name="w_t")
            b_t = sb.tile([128, 9], F32, name="b_t")
            nc.sync.dma_start(out=w_t[:], in_=wts[:])
            nc.sync.dma_start(out=b_t[:], in_=bias[:])
            for t in range(8):
                seg = PXC // 8
                x_t = sb.tile([C, seg], F32R, tag="x", name="x_t")
                nc.sync.dma_start(out=x_t[:], in_=xg[:, t * seg:(t + 1) * seg])
                nc.sync.dma_start(out=og[:, t * seg:(t + 1) * seg],
                                  in_=x_t[:].bitcast(F32))
    _split_multi_waits(nc, limit=1)
    return nc


def io_baseline(x, style, W0, b0, W1, b1, W2, b2, W3, b3, W4, b4):
    if "nc_io" not in _CACHE:
        _CACHE["nc_io"] = _build_io_baseline()
    nc = _CACHE["nc_io"]
    f32 = lambda a: np.ascontiguousarray(np.asarray(a), dtype=np.float32)
    x = f32(x)
    wts, bias = _pack_weights(f32(style), f32(W0), f32(b0), f32(W1), f32(b1),
                              f32(W2), f32(b2), f32(W3), f32(b3), f32(W4), f32(b4))
    xf = x.reshape(N, C, H * W)
    in_maps = []
    for core in range(N_CORES):
        n, j = divmod(core, 2)
        xc = np.ascontiguousarray(xf[n, :, j * PXC:(j + 1) * PXC])
        in_maps.append({"xg": xc, "wts": wts, "bias": bias})
    res = run_bass_kernel_spmd(nc, in_maps, list(range(N_CORES)))
    return res


def kernel(x, style, W0, b0, W1, b1, W2, b2, W3, b3, W4, b4,
           _want_results=False, _trace=False):
    if "nc" not in _CACHE:
        _CACHE["nc"] = _build_module()
    nc = _CACHE["nc"]

    f32 = lambda a: np.ascontiguousarray(np.asarray(a), dtype=np.float32)
    x = f32(x)
    wts, bias = _pack_weights(f32(style), f32(W0), f32(b0), f32(W1), f32(b1),
                              f32(W2), f32(b2), f32(W3), f32(b3), f32(W4), f32(b4))

    # [4,3,512,512] -> per-core [3, 131072]: core c=2n+j takes image n, hw-half j
    xf = x.reshape(N, C, H * W)
    in_maps = []
    for core in range(N_CORES):
        n, j = divmod(core, 2)
        xc = np.ascontiguousarray(xf[n, :, j * PXC:(j + 1) * PXC])
        in_maps.append({"xg": xc, "wts": wts, "bias": bias})

    res = run_bass_kernel_spmd(nc, in_maps, list(range(N_CORES)), trace=_trace)

    out = np.empty((N, C, H * W), dtype=np.float32)
    for core in range(N_CORES):
        n, j = divmod(core, 2)
        out[n, :, j * PXC:(j + 1) * PXC] = res.results[core]["og"]
    out = out.reshape(N, C, H, W)
    if _want_results:
        return out, res
    return out


# revision 16
# speedup vs baseline: 2.7367x; 2.7367x over previous
"""Trainium2 Bass kernel for nn_CNILUT: per-pixel MLP (3->256->256->256->256->3)
with relu/tanh activations and residual clamp, data-parallel over 8 NeuronCores.

Strategy:
- Shard the flattened pixel axis (n*h*w = 1,048,576 px) across 8 cores
  (131,072 px each); replicate the tiny MLP weights.
- Feature-major dataflow: activations live as [features(partitions), pixels]
  which is exactly the channel-planar layout of x, so no transposes anywhere.
- style is folded into layer-0's bias on the host (b0_eff = b0 + style@W0[3:6]),
  so layer 0 is a K=3 matmul over the 3 image channels only.
- Matmuls run as float32r (TF32-like, 1 cycle/row vs 4 for fp32; rel err ~2e-4).
- tanh (+bias) on ScalarE directly from PSUM; relu (+bias) and the final
  residual-add + clamp on VectorE.
"""

import os
import sys

for _p in ("/opt/trn_rl_repo", "/root/.axon_site/_ro/trn_rl_repo"):
    if os.path.isdir(_p) and _p not in sys.path:
        sys.path.insert(0, _p)

import numpy as np

import concourse.bass as bass
import concourse.tile as tile
from concourse import mybir
from concourse.bass_utils import run_bass_kernel_spmd

F32 = mybir.dt.float32
F32R = mybir.dt.float32r

N_CORES = 8
N, C, H, W = 4, 3, 512, 512
NF = 256
PXC = (N * H * W) // N_CORES  # pixels per core = 131072
T = 1024                      # pixels per tile
NT = PXC // T                 # 128 tiles per core

# packed weight layout (columns of the [128, WCOLS] f32r "wts" input)
# W{l}k{k} for hidden layers l=1..3 at (l-1)*512 + k*256, each [128, 256]
W4_OFF = 3 * 512              # W4k0 [128,3], W4k1 [128,3]
W0_OFF = W4_OFF + 6           # W0_eff [3, 256] on partitions 0..2
WCOLS = W0_OFF + 256

_CACHE = {}


S = 512                       # compute granularity (pixels) = one PSUM bank
D = 2048                      # DMA granularity (pixels)


def _build_module(nt=NT, split_waits=True, detect_races=True, reps=1,
                  psum_bufs=8, z_bufs=3):
    pxc = nt * T
    nd = pxc // D
    nc = bass.Bass(detect_race_conditions=detect_races)
    xg = nc.declare_dram_parameter("xg", [C, pxc], F32R, isOutput=False)
    wts = nc.declare_dram_parameter("wts", [128, WCOLS], F32R, isOutput=False)
    bias = nc.declare_dram_parameter("bias", [128, 9], F32, isOutput=False)
    og = nc.declare_dram_parameter("og", [C, pxc], F32, isOutput=True)

    TANH = mybir.ActivationFunctionType.Tanh
    ADD = mybir.AluOpType.add
    MAX = mybir.AluOpType.max
    MIN = mybir.AluOpType.min

    with tile.TileContext(nc) as tc:
        with tc.tile_pool(name="const", bufs=1) as const, \
             tc.tile_pool(name="io", bufs=3) as io, \
             tc.tile_pool(name="zs", bufs=z_bufs) as zs, \
             tc.tile_pool(name="ps", bufs=psum_bufs, space="PSUM") as ps:
            w_t = const.tile([128, WCOLS], F32R)
            b_t = const.tile([128, 9], F32)
            nc.sync.dma_start(out=w_t[:], in_=wts[:])
            nc.sync.dma_start(out=b_t[:], in_=bias[:])

            def lw(l, k, m):  # lhsT AP for hidden layer l (1..3), k/m chunks
                base = (l - 1) * 512 + k * 256
                return w_t[:, base + 128 * m: base + 128 * (m + 1)]

            # Software-pipelined emission: per-engine queues execute in
            # program order, so a flat per-tile loop stalls every engine on
            # the serial layer chain. Instead each "step" emits stage
            # L4(s-4), L3(s-3), L2(s-2), L1(s-1), L0(s) for five different
            # 512-px subtiles — every instruction's dependencies were
            # produced a full step earlier, and all engines stay busy.
            nsub_1 = nd * (D // S)          # subtiles per rep
            subs = [ss for _ in range(reps) for ss in range(nsub_1)]
            nsub = len(subs)
            SPD = D // S                    # subtiles per DMA tile
            xt = {}                         # live x_t D-tiles (by step idx)
            ot = {}
            zt = {}                         # z tiles: (step, layer, m)

            def xslice(i):
                return xt[i // SPD][:, (i % SPD) * S:(i % SPD + 1) * S]

            for step in range(nsub + 4):
                # stage L4 + finals for subtile step-4
                i = step - 4
                if 0 <= i < nsub:
                    p4 = ps.tile([3, S], F32, tag="p", name="p4")
                    for k in range(2):
                        nc.tensor.matmul(
                            p4[:], w_t[:, W4_OFF + 3 * k: W4_OFF + 3 * (k + 1)],
                            zt.pop((i, 3, k))[:], start=(k == 0), stop=(k == 1))
                    os_ = ot[i // SPD][:, (i % SPD) * S:(i % SPD + 1) * S]
                    nc.vector.scalar_tensor_tensor(
                        os_, p4[:], b_t[0:3, 8:9], xslice(i), ADD, ADD)
                    nc.vector.tensor_scalar(os_, os_, 0.0, 1.0, MAX, MIN)
                    if i % SPD == SPD - 1:
                        dd = subs[i] // SPD
                        nc.sync.dma_start(
                            out=og[:, dd * D:(dd + 1) * D], in_=ot[i // SPD][:])
                        del ot[i // SPD], xt[i // SPD]

                # stages L3, L2, L1 for subtiles step-3 .. step-1
                for l in (3, 2, 1):
                    i = step - l
                    if 0 <= i < nsub:
                        for m in range(2):
                            pN = ps.tile([128, S], F32, tag="p", name=f"p{l}_{m}")
                            for k in range(2):
                                nc.tensor.matmul(
                                    pN[:], lw(l, k, m), zt[(i, l - 1, k)][:],
                                    start=(k == 0), stop=(k == 1))
                            zm = zs.tile([128, S], F32R, tag=f"z{l}{m}",
                                         name=f"z{l}{m}")
                            nc.scalar.activation(
                                zm[:], pN[:], TANH,
                                bias=b_t[:, 2 * l + m:2 * l + m + 1], scale=1.0)
                            zt[(i, l, m)] = zm
                        for m in range(2):
                            zt.pop((i, l - 1, m))

                # stage L0 for subtile step (+ input DMA per D-tile)
                i = step
                if i < nsub:
                    if i % SPD == 0:
                        dd = subs[i] // SPD
                        x_t = io.tile([C, D], F32R, tag="x", name="x_t")
                        nc.sync.dma_start(out=x_t[:], in_=xg[:, dd * D:(dd + 1) * D])
                        xt[i // SPD] = x_t
                        ot[i // SPD] = io.tile([C, D], F32, tag="o", name="o_t")
                    for m in range(2):
                        p0 = ps.tile([128, S], F32, tag="p", name=f"p0_{m}")
                        nc.tensor.matmul(
                            p0[:],
                            w_t[0:3, W0_OFF + 128 * m: W0_OFF + 128 * (m + 1)],
                            xslice(i), start=True, stop=True)
                        zm = zs.tile([128, S], F32R, tag=f"z0{m}", name=f"z0{m}")
                        nc.vector.tensor_scalar(
                            zm[:], p0[:], b_t[:, m:m + 1], 0.0, ADD, MAX)
                        zt[(i, 0, m)] = zm

    if split_waits:
        _split_multi_waits(nc, limit=1)
    return nc


def _split_multi_waits(nc, limit=1):
    """walrus codegen on this toolchain accepts only `limit` sync waits per
    instruction (the fused f32r matmul's LDWEIGHTS slot takes just one).
    Tile attaches N waits freely; split the extras onto single-wait NoOps
    immediately preceding, on the same engine — semantics preserving since
    an engine queue executes in order."""
    n = 0
    for fn in nc.m.functions:
        for bb in fn.blocks:
            insts = bb.instructions
            out = []
            changed = False
            for inst in insts:
                si = inst.sync_info
                if si is not None and si.on_wait and len(si.on_wait) > limit:
                    waits = list(si.on_wait)
                    for j, w in enumerate(waits[:-limit]):
                        nop = mybir.InstNoOp(name=f"{inst.name}-wsplit{j}")
                        nop.engine = inst.engine
                        nop.sync_info = mybir.SyncInfo(on_wait=[w], on_update=[])
                        out.append(nop)
                        n += 1
                    inst.sync_info = mybir.SyncInfo(
                        on_wait=waits[-limit:], on_update=list(si.on_update))
                    changed = True
                out.append(inst)
            if changed:
                insts.clear()
                insts.extend(out)
    return n


def _pack_weights(style, W0, b0, W1, b1, W2, b2, W3, b3, W4, b4):
    w = np.zeros((128, WCOLS), dtype=np.float32)
    for l, Wl in ((1, W1), (2, W2), (3, W3)):
        base = (l - 1) * 512
        w[:, base:base + 256] = Wl[0:128, :]
        w[:, base + 256:base + 512] = Wl[128:256, :]
    w[:, W4_OFF:W4_OFF + 3] = W4[0:128, :]
    w[:, W4_OFF + 3:W4_OFF + 6] = W4[128:256, :]
    w[0:3, W0_OFF:W0_OFF + 256] = W0[0:3, :]

    b0_eff = b0 + style @ W0[3:6, :]
    b = np.zeros((128, 9), dtype=np.float32)
    for i, bl in enumerate((b0_eff, b1, b2, b3)):
        b[:, 2 * i] = bl[0:128]
        b[:, 2 * i + 1] = bl[128:256]
    b[0:3, 8] = b4
    return w, b


def _build_io_baseline():
    """Same external IO as the real kernel, but pure DMA passthrough —
    used by test.py to subtract host<->device transfer overhead from
    wall-clock timings."""
    nc = bass.Bass()
    xg = nc.declare_dram_parameter("xg", [C, PXC], F32R, isOutput=False)
    wts = nc.declare_dram_parameter("wts", [128, WCOLS], F32R, isOutput=False)
    bias = nc.declare_dram_parameter("bias", [128, 9], F32, isOutput=False)
    og = nc.declare_dram_parameter("og", [C, PXC], F32, isOutput=True)
    with tile.TileContext(nc) as tc:
        with tc.tile_pool(name="sb", bufs=2) as sb:
            w_t = sb.tile([128, WCOLS], F32R, name="w_t")
            b_t = sb.tile([128, 9], F32, name="b_t")
            nc.sync.dma_start(out=w_t[:], in_=wts[:])
            nc.sync.dma_start(out=b_t[:], in_=bias[:])
            for t in range(8):
                seg = PXC // 8
                x_t = sb.tile([C, seg], F32R, tag="x", name="x_t")
                nc.sync.dma_start(out=x_t[:], in_=xg[:, t * seg:(t + 1) * seg])
                nc.sync.dma_start(out=og[:, t * seg:(t + 1) * seg],
                                  in_=x_t[:].bitcast(F32))
    _split_multi_waits(nc, limit=1)
    return nc


def io_baseline(x, style, W0, b0, W1, b1, W2, b2, W3, b3, W4, b4):
    if "nc_io" not in _CACHE:
        _CACHE["nc_io"] = _build_io_baseline()
    nc = _CACHE["nc_io"]
    f32 = lambda a: np.ascontiguousarray(np.asarray(a), dtype=np.float32)
    x = f32(x)
    wts, bias = _pack_weights(f32(style), f32(W0), f32(b0), f32(W1), f32(b1),
                              f32(W2), f32(b2), f32(W3), f32(b3), f32(W4), f32(b4))
    xf = x.reshape(N, C, H * W)
    in_maps = []
    for core in range(N_CORES):
        n, j = divmod(core, 2)
        xc = np.ascontiguousarray(xf[n, :, j * PXC:(j + 1) * PXC])
        in_maps.append({"xg": xc, "wts": wts, "bias": bias})
    res = run_bass_kernel_spmd(nc, in_maps, list(range(N_CORES)))
    return res


def kernel(x, style, W0, b0, W1, b1, W2, b2, W3, b3, W4, b4,
           _want_results=False, _trace=False):
    if "nc" not in _CACHE:
        _CACHE["nc"] = _build_module()
    nc = _CACHE["nc"]

    f32 = lambda a: np.ascontiguousarray(np.asarray(a), dtype=np.float32)
    x = f32(x)
    wts, bias = _pack_weights(f32(style), f32(W0), f32(b0), f32(W1), f32(b1),
                              f32(W2), f32(b2), f32(W3), f32(b3), f32(W4), f32(b4))

    # [4,3,512,512] -> per-core [3, 131072]: core c=2n+j takes image n, hw-half j
    xf = x.reshape(N, C, H * W)
    in_maps = []
    for core in range(N_CORES):
        n, j = divmod(core, 2)
        xc = np.ascontiguousarray(xf[n, :, j * PXC:(j + 1) * PXC])
        in_maps.append({"xg": xc, "wts": wts, "bias": bias})

    res = run_bass_kernel_spmd(nc, in_maps, list(range(N_CORES)), trace=_trace)

    out = np.empty((N, C, H * W), dtype=np.float32)
    for core in range(N_CORES):
        n, j = divmod(core, 2)
        out[n, :, j * PXC:(j + 1) * PXC] = res.results[core]["og"]
    out = out.reshape(N, C, H, W)
    if _want_results:
        return out, res
    return out


# revision 18
# speedup vs baseline: 3.0197x; 1.1034x over previous
"""Trainium2 Bass kernel for nn_CNILUT: per-pixel MLP (3->256->256->256->256->3)
with relu/tanh activations and residual clamp, data-parallel over 8 NeuronCores.

Strategy:
- Shard the flattened pixel axis (n*h*w = 1,048,576 px) across 8 cores
  (131,072 px each); replicate the tiny MLP weights.
- Feature-major dataflow: activations live as [features(partitions), pixels]
  which is exactly the channel-planar layout of x, so no transposes anywhere.
- style is folded into layer-0's bias on the host (b0_eff = b0 + style@W0[3:6]),
  so layer 0 is a K=3 matmul over the 3 image channels only.
- Matmuls run as float32r (TF32-like, 1 cycle/row vs 4 for fp32; rel err ~2e-4).
- tanh (+bias) on ScalarE directly from PSUM; relu (+bias) and the final
  residual-add + clamp on VectorE.
"""

import os
import sys

for _p in ("/opt/trn_rl_repo", "/root/.axon_site/_ro/trn_rl_repo"):
    if os.path.isdir(_p) and _p not in sys.path:
        sys.path.insert(0, _p)

import numpy as np

import concourse.bass as bass
import concourse.tile as tile
from concourse import mybir
from concourse.bass_utils import run_bass_kernel_spmd

F32 = mybir.dt.float32
F32R = mybir.dt.float32r

N_CORES = 8
N, C, H, W = 4, 3, 512, 512
NF = 256
PXC = (N * H * W) // N_CORES  # pixels per core = 131072
T = 1024                      # pixels per tile
NT = PXC // T                 # 128 tiles per core

# packed weight layout (columns of the [128, WCOLS] f32r "wts" input)
# W{l}k{k} for hidden layers l=1..3 at (l-1)*512 + k*256, each [128, 256]
W4_OFF = 3 * 512              # W4k0 [128,3], W4k1 [128,3]
W0_OFF = W4_OFF + 6           # W0_eff [3, 256] on partitions 0..2
WCOLS = W0_OFF + 256

_CACHE = {}


S = 512                       # compute granularity (pixels) = one PSUM bank
D = 2048                      # DMA granularity (pixels)


def _build_module(nt=NT, split_waits=True, detect_races=True, reps=1,
                  psum_bufs=None, z_bufs=3, s=S):
    pxc = nt * T
    nd = pxc // D
    nh = s // 512                  # matmul N=512 chunks per psum tile
    if psum_bufs is None:
        psum_bufs = 8 // nh
    nc = bass.Bass(detect_race_conditions=detect_races)
    xg = nc.declare_dram_parameter("xg", [C, pxc], F32R, isOutput=False)
    wts = nc.declare_dram_parameter("wts", [128, WCOLS], F32R, isOutput=False)
    bias = nc.declare_dram_parameter("bias", [128, 9], F32, isOutput=False)
    og = nc.declare_dram_parameter("og", [C, pxc], F32, isOutput=True)

    TANH = mybir.ActivationFunctionType.Tanh
    ADD = mybir.AluOpType.add
    MAX = mybir.AluOpType.max
    MIN = mybir.AluOpType.min

    with tile.TileContext(nc) as tc:
        with tc.tile_pool(name="const", bufs=1) as const, \
             tc.tile_pool(name="io", bufs=3) as io, \
             tc.tile_pool(name="zs", bufs=z_bufs) as zs, \
             tc.tile_pool(name="ps", bufs=psum_bufs, space="PSUM") as ps:
            w_t = const.tile([128, WCOLS], F32R)
            b_t = const.tile([128, 9], F32)
            nc.sync.dma_start(out=w_t[:], in_=wts[:])
            nc.sync.dma_start(out=b_t[:], in_=bias[:])

            def lw(l, k, m):  # lhsT AP for hidden layer l (1..3), k/m chunks
                base = (l - 1) * 512 + k * 256
                return w_t[:, base + 128 * m: base + 128 * (m + 1)]

            # Software-pipelined emission: per-engine queues execute in
            # program order, so a flat per-tile loop stalls every engine on
            # the serial layer chain. Instead each "step" emits stage
            # L4(s-4), L3(s-3), L2(s-2), L1(s-1), L0(s) for five different
            # 512-px subtiles — every instruction's dependencies were
            # produced a full step earlier, and all engines stay busy.
            nsub_1 = nd * (D // s)          # subtiles per rep
            subs = [ss for _ in range(reps) for ss in range(nsub_1)]
            nsub = len(subs)
            SPD = D // s                    # subtiles per DMA tile
            HS = [(h * 512, (h + 1) * 512) for h in range(nh)]
            xt = {}                         # live x_t D-tiles (by step idx)
            ot = {}
            zt = {}                         # z tiles: (step, layer, m)

            def xslice(i):
                return xt[i // SPD][:, (i % SPD) * s:(i % SPD + 1) * s]

            for step in range(nsub + 4):
                # stage L4 + finals for subtile step-4
                i = step - 4
                if 0 <= i < nsub:
                    p4 = ps.tile([3, s], F32, tag="p", name="p4")
                    z3 = [zt.pop((i, 3, k)) for k in range(2)]
                    for h0, h1 in HS:
                        for k in range(2):
                            nc.tensor.matmul(
                                p4[:, h0:h1],
                                w_t[:, W4_OFF + 3 * k: W4_OFF + 3 * (k + 1)],
                                z3[k][:, h0:h1], start=(k == 0), stop=(k == 1))
                    os_ = ot[i // SPD][:, (i % SPD) * s:(i % SPD + 1) * s]
                    nc.vector.scalar_tensor_tensor(
                        os_, p4[:], b_t[0:3, 8:9], xslice(i), ADD, ADD)
                    nc.vector.tensor_scalar(os_, os_, 0.0, 1.0, MAX, MIN)
                    if i % SPD == SPD - 1:
                        dd = subs[i] // SPD
                        nc.sync.dma_start(
                            out=og[:, dd * D:(dd + 1) * D], in_=ot[i // SPD][:])
                        del ot[i // SPD], xt[i // SPD]

                # stages L3, L2, L1 for subtiles step-3 .. step-1
                for l in (3, 2, 1):
                    i = step - l
                    if 0 <= i < nsub:
                        for m in range(2):
                            pN = ps.tile([128, s], F32, tag="p", name=f"p{l}_{m}")
                            for h0, h1 in HS:
                                for k in range(2):
                                    nc.tensor.matmul(
                                        pN[:, h0:h1], lw(l, k, m),
                                        zt[(i, l - 1, k)][:, h0:h1],
                                        start=(k == 0), stop=(k == 1))
                            zm = zs.tile([128, s], F32R, tag=f"z{l}{m}",
                                         name=f"z{l}{m}")
                            nc.scalar.activation(
                                zm[:], pN[:], TANH,
                                bias=b_t[:, 2 * l + m:2 * l + m + 1], scale=1.0)
                            zt[(i, l, m)] = zm
                        for m in range(2):
                            zt.pop((i, l - 1, m))

                # stage L0 for subtile step (+ input DMA per D-tile)
                i = step
                if i < nsub:
                    if i % SPD == 0:
                        dd = subs[i] // SPD
                        x_t = io.tile([C, D], F32R, tag="x", name="x_t")
                        nc.sync.dma_start(out=x_t[:], in_=xg[:, dd * D:(dd + 1) * D])
                        xt[i // SPD] = x_t
                        ot[i // SPD] = io.tile([C, D], F32, tag="o", name="o_t")
                    xs_ = xslice(i)
                    for m in range(2):
                        p0 = ps.tile([128, s], F32, tag="p", name=f"p0_{m}")
                        for h0, h1 in HS:
                            nc.tensor.matmul(
                                p0[:, h0:h1],
                                w_t[0:3, W0_OFF + 128 * m: W0_OFF + 128 * (m + 1)],
                                xs_[:, h0:h1], start=True, stop=True)
                        zm = zs.tile([128, s], F32R, tag=f"z0{m}", name=f"z0{m}")
                        nc.vector.tensor_scalar(
                            zm[:], p0[:], b_t[:, m:m + 1], 0.0, ADD, MAX)
                        zt[(i, 0, m)] = zm

    if split_waits:
        _split_multi_waits(nc, limit=1)
    return nc


def _split_multi_waits(nc, limit=1):
    """walrus codegen on this toolchain accepts only `limit` sync waits per
    instruction (the fused f32r matmul's LDWEIGHTS slot takes just one).
    Tile attaches N waits freely; split the extras onto single-wait NoOps
    immediately preceding, on the same engine — semantics preserving since
    an engine queue executes in order."""
    n = 0
    for fn in nc.m.functions:
        for bb in fn.blocks:
            insts = bb.instructions
            out = []
            changed = False
            for inst in insts:
                si = inst.sync_info
                if si is not None and si.on_wait and len(si.on_wait) > limit:
                    waits = list(si.on_wait)
                    for j, w in enumerate(waits[:-limit]):
                        nop = mybir.InstNoOp(name=f"{inst.name}-wsplit{j}")
                        nop.engine = inst.engine
                        nop.sync_info = mybir.SyncInfo(on_wait=[w], on_update=[])
                        out.append(nop)
                        n += 1
                    inst.sync_info = mybir.SyncInfo(
                        on_wait=waits[-limit:], on_update=list(si.on_update))
                    changed = True
                out.append(inst)
            if changed:
                insts.clear()
                insts.extend(out)
    return n


def _pack_weights(style, W0, b0, W1, b1, W2, b2, W3, b3, W4, b4):
    w = np.zeros((128, WCOLS), dtype=np.float32)
    for l, Wl in ((1, W1), (2, W2), (3, W3)):
        base = (l - 1) * 512
        w[:, base:base + 256] = Wl[0:128, :]
        w[:, base + 256:base + 512] = Wl[128:256, :]
    w[:, W4_OFF:W4_OFF + 3] = W4[0:128, :]
    w[:, W4_OFF + 3:W4_OFF + 6] = W4[128:256, :]
    w[0:3, W0_OFF:W0_OFF + 256] = W0[0:3, :]

    b0_eff = b0 + style @ W0[3:6, :]
    b = np.zeros((128, 9), dtype=np.float32)
    for i, bl in enumerate((b0_eff, b1, b2, b3)):
        b[:, 2 * i] = bl[0:128]
        b[:, 2 * i + 1] = bl[128:256]
    b[0:3, 8] = b4
    return w, b


def _build_io_baseline():
    """Same external IO as the real kernel, but pure DMA passthrough —
    used by test.py to subtract host<->device transfer overhead from
    wall-clock timings."""
    nc = bass.Bass()
    xg = nc.declare_dram_parameter("xg", [C, PXC], F32R, isOutput=False)
    wts = nc.declare_dram_parameter("wts", [128, WCOLS], F32R, isOutput=False)
    bias = nc.declare_dram_parameter("bias", [128, 9], F32, isOutput=False)
    og = nc.declare_dram_parameter("og", [C, PXC], F32, isOutput=True)
    with tile.TileContext(nc) as tc:
        with tc.tile_pool(name="sb", bufs=2) as sb:
            w_t = sb.tile([128, WCOLS], F32R, name="w_t")
            b_t = sb.tile([128, 9], F32, name="b_t")
            nc.sync.dma_start(out=w_t[:], in_=wts[:])
            nc.sync.dma_start(out=b_t[:], in_=bias[:])
            for t in range(8):
                seg = PXC // 8
                x_t = sb.tile([C, seg], F32R, tag="x", name="x_t")
                nc.sync.dma_start(out=x_t[:], in_=xg[:, t * seg:(t + 1) * seg])
                nc.sync.dma_start(out=og[:, t * seg:(t + 1) * seg],
                                  in_=x_t[:].bitcast(F32))
    _split_multi_waits(nc, limit=1)
    return nc


def io_baseline(x, style, W0, b0, W1, b1, W2, b2, W3, b3, W4, b4):
    if "nc_io" not in _CACHE:
        _CACHE["nc_io"] = _build_io_baseline()
    nc = _CACHE["nc_io"]
    f32 = lambda a: np.ascontiguousarray(np.asarray(a), dtype=np.float32)
    x = f32(x)
    wts, bias = _pack_weights(f32(style), f32(W0), f32(b0), f32(W1), f32(b1),
                              f32(W2), f32(b2), f32(W3), f32(b3), f32(W4), f32(b4))
    xf = x.reshape(N, C, H * W)
    in_maps = []
    for core in range(N_CORES):
        n, j = divmod(core, 2)
        xc = np.ascontiguousarray(xf[n, :, j * PXC:(j + 1) * PXC])
        in_maps.append({"xg": xc, "wts": wts, "bias": bias})
    res = run_bass_kernel_spmd(nc, in_maps, list(range(N_CORES)))
    return res


def kernel(x, style, W0, b0, W1, b1, W2, b2, W3, b3, W4, b4,
           _want_results=False, _trace=False):
    if "nc" not in _CACHE:
        _CACHE["nc"] = _build_module()
    nc = _CACHE["nc"]

    f32 = lambda a: np.ascontiguousarray(np.asarray(a), dtype=np.float32)
    x = f32(x)
    wts, bias = _pack_weights(f32(style), f32(W0), f32(b0), f32(W1), f32(b1),
                              f32(W2), f32(b2), f32(W3), f32(b3), f32(W4), f32(b4))

    # [4,3,512,512] -> per-core [3, 131072]: core c=2n+j takes image n, hw-half j
    xf = x.reshape(N, C, H * W)
    in_maps = []
    for core in range(N_CORES):
        n, j = divmod(core, 2)
        xc = np.ascontiguousarray(xf[n, :, j * PXC:(j + 1) * PXC])
        in_maps.append({"xg": xc, "wts": wts, "bias": bias})

    res = run_bass_kernel_spmd(nc, in_maps, list(range(N_CORES)), trace=_trace)

    out = np.empty((N, C, H * W), dtype=np.float32)
    for core in range(N_CORES):
        n, j = divmod(core, 2)
        out[n, :, j * PXC:(j + 1) * PXC] = res.results[core]["og"]
    out = out.reshape(N, C, H, W)
    if _want_results:
        return out, res
    return out
